# revision 12
# baseline (speedup 1.0000x reference)
"""BankedLinear (MoE-style banked linear) Trainium2 Bass kernel.

Math: out[n] = sum_k bank_weights[n,k] * (tensor[n] @ W[sel[n,k]] + bias[sel[n,k]])
Shapes: tensor [8192,128] f32, bank_weights [8192,2] f32, bank_selections [8192,2] int,
        weights [64,128,128] f32, bias [64,128] f32 -> out [8192,128] f32.

Strategy (data parallel over tokens, weights replicated, fp16 compute):
  - 8 cores x 1024 tokens, greedily assigned so per-bank pair counts are
    balanced across cores (SPMD: one program, shared per-bank capacity plan).
  - Two sorted passes per core. Pass A processes each token's k=0 pair with
    tokens sorted by sel[:,0]; pass B processes k=1 pairs sorted by sel[:,1].
    The host pre-builds x^T column tiles in each sorted order with the
    bank_weight folded in (column s = bw[pair s] * x[token s], fp16), so a
    single fp16 matmul per bank segment computes bw*x@W. The bias term is
    added per 512-column psum chunk with one 64-contraction matmul against a
    one-hot routing matrix sm[b, s] = bw[s]*[bank(s)==b] (the aux tensor
    packs sm0/sm1 on partition halves plus a duplicated bias block). Each
    chunk is one psum accumulation group: start=True only on its first
    matmul, stop=True on the closing bias matmul.
  - psum chunk -> fp16 SBUF evict (DVE / Pool), then an xbar DMA transpose
    turns the [out, slot] columns into row-layout tiles (row j of the slot
    order lands at [j % 128, j // 128]).
  - Pass A rows land in slot order = output order: contiguous DMA to the
    fp16 out tensor. Pass B rows are combined with a single SWDGE
    dma_scatter_add (out[idx[j]] += rows1[j]) whose per-core index tensor
    maps each k=1 slot to its token's k=0 slot. Pad rows are exactly zero
    and are pointed at pass-A pad slots (never at a real row: on hardware
    the DRAM read-modify-write of a pad's +0 add can race a real add).
  - Host unshards: out[assign[c][i]] = dev_out[c][pos0[c][i]], upcast f32.
"""

import numpy as np

N, K, IN, OUT, NUM_BANKS = 8192, 2, 128, 128, 64
NCORES = 8
NLOC = N // NCORES
P = 128
PSUM_FREE = 512


def _routing_plan(sel_all):
    """Greedy token->core assignment balancing per-bank counts for both the
    k=0 and k=1 pair distributions. Returns (assign, caps0, offs0, caps1,
    offs1, Cap) with a shared capacity plan (SPMD) and equal Cap per pass."""
    sel_all = np.asarray(sel_all).astype(np.int64)
    g0 = np.bincount(sel_all[:, 0], minlength=NUM_BANKS)
    g1 = np.bincount(sel_all[:, 1], minlength=NUM_BANKS)
    ideal0 = (g0 + NCORES - 1) // NCORES
    ideal1 = (g1 + NCORES - 1) // NCORES
    c0 = np.zeros((NCORES, NUM_BANKS), dtype=np.int64)
    c1 = np.zeros((NCORES, NUM_BANKS), dtype=np.int64)
    fill = np.zeros(NCORES, dtype=np.int64)
    assign_lists = [[] for _ in range(NCORES)]
    for n in range(N):
        b0, b1 = int(sel_all[n, 0]), int(sel_all[n, 1])
        best, best_key = -1, None
        for c in range(NCORES):
            if fill[c] >= NLOC:
                continue
            over = max(0, c0[c, b0] + 1 - ideal0[b0]) + \
                max(0, c1[c, b1] + 1 - ideal1[b1])
            key = (over, c0[c, b0] + c1[c, b1], fill[c])
            if best < 0 or key < best_key:
                best, best_key = c, key
        c0[best, b0] += 1
        c1[best, b1] += 1
        fill[best] += 1
        assign_lists[best].append(n)
    assign = np.array(assign_lists, dtype=np.int64)

    caps0 = c0.max(axis=0).astype(np.int64)
    caps1 = c1.max(axis=0).astype(np.int64)
    Cap = -(-max(int(caps0.sum()), int(caps1.sum())) // P) * P

    def pad_to(caps, target):
        pad = target - int(caps.sum())
        for i in range(pad):
            caps[i % NUM_BANKS] += 1
        offs = np.concatenate([[0], np.cumsum(caps)[:-1]]).astype(np.int64)
        return caps, offs

    caps0, offs0 = pad_to(caps0, Cap)
    caps1, offs1 = pad_to(caps1, Cap)
    return assign, caps0, offs0, caps1, offs1, Cap


def _wrap_idx(flat_idx):
    """Wrap a flat int16 index list into the [128, n//16] SWDGE layout."""
    n = flat_idx.shape[0]
    assert n % 16 == 0
    w = flat_idx.reshape(n // 16, 16).T.astype(np.int16)
    return np.tile(w, (8, 1))


def _segments(caps, offs):
    """Per-bank psum column segments split at PSUM_FREE boundaries, grouped
    by chunk. Returns {chunk: [(bank, col_start, width), ...]}."""
    by_chunk = {}
    for b in range(NUM_BANKS):
        cb, ob = int(caps[b]), int(offs[b])
        while cb > 0:
            room = PSUM_FREE - (ob % PSUM_FREE)
            w = min(cb, room)
            by_chunk.setdefault(ob // PSUM_FREE, []).append((b, ob, w))
            ob += w
            cb -= w
    return by_chunk


def _build_program(caps0, offs0, caps1, offs1, Cap):
    import concourse.bacc as bacc
    import concourse.tile as tile
    from concourse import mybir, library_config
    from concourse.tile import add_dep_helper

    f32 = mybir.dt.float32
    f16 = mybir.dt.float16
    i16 = mybir.dt.int16

    nblk = Cap // P
    nch = (Cap + PSUM_FREE - 1) // PSUM_FREE

    nc = bacc.Bacc(None, target_bir_lowering=False, debug=False)

    x0_d = nc.declare_dram_parameter("x0", [P, Cap], f16, isOutput=False)
    x1_d = nc.declare_dram_parameter("x1", [P, Cap], f16, isOutput=False)
    w_d = nc.declare_dram_parameter("wts", [P, NUM_BANKS * OUT], f16,
                                    isOutput=False)
    aux_d = nc.declare_dram_parameter("aux", [P, Cap + P], f16, isOutput=False)
    idx_d = nc.declare_dram_parameter("sidx", [P, Cap // 16], i16,
                                      isOutput=False)
    out_d = nc.declare_dram_parameter("out", [Cap, OUT], f16, isOutput=True)

    ch0 = _segments(caps0, offs0)
    ch1 = _segments(caps1, offs1)

    # weight DMA chunks split at the pass-A psum chunk boundaries so chunk
    # k's matmuls only wait for weight chunk k
    wcut = [0]
    for i in range(1, nch):
        b = next(b for b in range(NUM_BANKS) if offs0[b] >= i * PSUM_FREE)
        wcut.append(b)
    wcut.append(NUM_BANKS)

    with tile.TileContext(nc) as tc:
        with (
            tc.tile_pool(name="const", bufs=1) as cpool,
            tc.tile_pool(name="big", bufs=1) as bigpool,
            tc.tile_pool(name="psA", bufs=1, space="PSUM") as psA,
            tc.tile_pool(name="psB", bufs=1, space="PSUM") as psB,
        ):
            libload = nc.gpsimd.load_library(library_config.mlp)

            # input DMAs on SP; weight chunks on ACT/ACT/Pool
            aux_sb = cpool.tile([P, Cap + P], f16)
            nc.sync.dma_start(out=aux_sb[:], in_=aux_d.ap())
            xs0 = cpool.tile([P, Cap], f16)
            nc.sync.dma_start(out=xs0[:], in_=x0_d.ap())
            xs1 = cpool.tile([P, Cap], f16)
            nc.sync.dma_start(out=xs1[:], in_=x1_d.ap())
            idx_sb = cpool.tile([P, Cap // 16], i16)
            nc.sync.dma_start(out=idx_sb[:], in_=idx_d.ap())

            w_sb = cpool.tile([P, NUM_BANKS * OUT], f16)
            wengs = [nc.scalar, nc.scalar, nc.gpsimd]
            for i in range(nch):
                lo, hi = wcut[i] * OUT, wcut[i + 1] * OUT
                wengs[min(i, 2)].dma_start(out=w_sb[:, lo:hi],
                                           in_=w_d[:, lo:hi])

            pA, pB = [], []
            for i in range(nch):
                pt = psA.tile([P, PSUM_FREE], f32, tag=f"pA{i}", name=f"pA{i}")
                pA.append(pt)
            for i in range(nch):
                pt = psB.tile([P, PSUM_FREE], f32, tag=f"pB{i}", name=f"pB{i}")
                pB.append(pt)

            y0T = bigpool.tile([P, Cap], f16, tag="y0T")
            y1T = bigpool.tile([P, Cap], f16, tag="y1T")
            rows0 = bigpool.tile([P, nblk, OUT], f16, tag="rows0")
            rows1 = bigpool.tile([P, nblk, OUT], f16, tag="rows1")

            def do_chunk(chsegs, ptile, xs, yT, rows, pbase, ci, is_a):
                lo = ci * PSUM_FREE
                wch = min(PSUM_FREE, Cap - lo)
                # one accumulation group per chunk (start on first matmul,
                # stop on the closing bias matmul)
                for j, (b, ob, wseg) in enumerate(chsegs):
                    nc.tensor.matmul(out=ptile[:, ob - lo:ob - lo + wseg],
                                     lhsT=w_sb[:, b * OUT:(b + 1) * OUT],
                                     rhs=xs[:, ob:ob + wseg],
                                     start=(j == 0), stop=False)
                nc.tensor.matmul(
                    out=ptile[:, :wch],
                    lhsT=aux_sb[pbase:pbase + NUM_BANKS, Cap:Cap + P],
                    rhs=aux_sb[pbase:pbase + NUM_BANKS, lo:lo + wch],
                    start=False, stop=True)
                # evict psum -> fp16 (DVE; Pool has no PSUM read port)
                nc.vector.tensor_copy(yT[:, lo:lo + wch], ptile[:, :wch])
                # xbar transpose to row layout (SP queue)
                blo = lo // P
                bn = wch // P
                nc.sync.dma_start_transpose(
                    out=rows[:, blo:blo + bn, :], in_=yT[:, lo:lo + wch])
                if is_a:
                    # contiguous out write for pass A (ACT queue)
                    nc.scalar.dma_start(
                        out=out_d[lo:lo + wch].rearrange("(t p) o -> p t o",
                                                         p=P),
                        in_=rows[:, blo:blo + bn, :])

            for ci in range(nch):
                do_chunk(ch0[ci], pA[ci], xs0, y0T, rows0, 0, ci, True)
            for ci in range(nch):
                do_chunk(ch1[ci], pB[ci], xs1, y1T, rows1, NUM_BANKS, ci,
                         False)

            sa = nc.gpsimd.dma_scatter_add(
                out_ap=out_d.ap(),
                in_ap=rows1[:, :, :],
                idxs_ap=idx_sb[:],
                num_idxs=Cap,
                num_idxs_reg=Cap,
                elem_size=OUT,
                single_packet=Cap <= 1024,
            )
            add_dep_helper(sa.ins, libload.ins, sync=False,
                           reason="scatter-add needs mlp gpsimd library")

    return nc


def _make_in_maps(tensor, bank_weights, bank_selections, weights, bias,
                  assign, caps0, offs0, caps1, offs1, Cap):
    tensor = np.ascontiguousarray(tensor, dtype=np.float32)
    bank_weights = np.ascontiguousarray(bank_weights, dtype=np.float32)
    sel_all = np.asarray(bank_selections).astype(np.int64)
    w16 = np.ascontiguousarray(
        np.asarray(weights, dtype=np.float32).transpose(1, 0, 2)
        .reshape(IN, NUM_BANKS * OUT)).astype(np.float16)
    bias16 = np.ascontiguousarray(bias, dtype=np.float32).astype(np.float16)

    in_maps = []
    pos0_all = []
    for c in range(NCORES):
        toks = assign[c]
        sel = sel_all[toks]                      # [NLOC, K]
        bw = bank_weights[toks]                  # [NLOC, K]
        x = tensor[toks]                         # [NLOC, IN]

        def lay(k, offs):
            slot = np.zeros(NLOC, dtype=np.int64)
            fillb = offs.copy()
            for i in range(NLOC):
                b = sel[i, k]
                slot[i] = fillb[b]
                fillb[b] += 1
            xbw = np.zeros((Cap, IN), dtype=np.float32)
            xbw[slot] = x * bw[:, k:k + 1]
            sm = np.zeros((NUM_BANKS, Cap), dtype=np.float32)
            sm[sel[:, k], slot] = bw[:, k]
            return slot, np.ascontiguousarray(xbw.T).astype(np.float16), \
                sm.astype(np.float16)

        slot0, x0, sm0 = lay(0, offs0)
        slot1, x1, sm1 = lay(1, offs1)
        pos0_all.append(slot0)

        aux = np.zeros((P, Cap + P), dtype=np.float16)
        aux[:NUM_BANKS, :Cap] = sm0
        aux[NUM_BANKS:, :Cap] = sm1
        aux[:NUM_BANKS, Cap:] = bias16
        aux[NUM_BANKS:, Cap:] = bias16

        # scatter indices: k=1 slot j -> that token's k=0 slot. Pad rows are
        # all-zero but their scatter-add is a DRAM read-modify-write that can
        # race a real add to the same row on hardware — point them at pass-A
        # pad slots (rows no token reads) instead of a shared real row.
        pad0 = np.setdiff1d(np.arange(Cap, dtype=np.int64), slot0)
        assert pad0.size > 0
        reps = (Cap + pad0.size - 1) // pad0.size
        sidx = np.tile(pad0, reps)[:Cap]
        sidx[slot1] = slot0
        in_maps.append({
            "x0": x0,
            "x1": x1,
            "wts": w16,
            "aux": aux,
            "sidx": _wrap_idx(sidx.astype(np.int16)),
        })
    return in_maps, pos0_all


def kernel(tensor, bank_weights, bank_selections, weights, bias):
    tensor = np.asarray(tensor)
    bank_weights = np.asarray(bank_weights)
    bank_selections = np.asarray(bank_selections)
    weights = np.asarray(weights)
    bias = np.asarray(bias)

    assign, caps0, offs0, caps1, offs1, Cap = _routing_plan(bank_selections)
    nc = _build_program(caps0, offs0, caps1, offs1, Cap)
    in_maps, pos0_all = _make_in_maps(
        tensor, bank_weights, bank_selections, weights, bias,
        assign, caps0, offs0, caps1, offs1, Cap)

    nc.finalize()
    from concourse.bass_utils import run_bass_kernel_spmd
    try:
        res = run_bass_kernel_spmd(nc, in_maps, list(range(NCORES)))
    except Exception:
        # one retry: a previous crashed session can leave the accelerator in
        # a transient bad state that clears on the next dispatch
        import time
        time.sleep(2.0)
        res = run_bass_kernel_spmd(nc, in_maps, list(range(NCORES)))
    out = np.empty((N, OUT), dtype=np.float32)
    for c in range(NCORES):
        out[assign[c]] = res.results[c]["out"][pos0_all[c]].astype(np.float32)
    return out


# revision 15
# speedup vs baseline: 1.1200x; 1.1200x over previous
"""BankedLinear (MoE-style banked linear) Trainium2 Bass kernel.

Math: out[n] = sum_k bank_weights[n,k] * (tensor[n] @ W[sel[n,k]] + bias[sel[n,k]])
Shapes: tensor [8192,128] f32, bank_weights [8192,2] f32, bank_selections [8192,2] int,
        weights [64,128,128] f32, bias [64,128] f32 -> out [8192,128] f32.

Strategy (data parallel over tokens, weights replicated, fp16 compute):
  - 8 cores x 1024 tokens, greedily assigned so per-bank pair counts are
    balanced across cores (SPMD: one program, shared per-bank capacity plan).
  - Two sorted passes per core. Pass A processes each token's k=0 pair with
    tokens sorted by sel[:,0]; pass B processes k=1 pairs sorted by sel[:,1].
    The host pre-builds x^T column tiles in each sorted order with the
    bank_weight folded in (column s = bw[pair s] * x[token s], fp16), so a
    single fp16 matmul per bank segment computes bw*x@W. The bias term is
    added per 512-column psum chunk with one 64-contraction matmul against a
    one-hot routing matrix sm[b, s] = bw[s]*[bank(s)==b] (the aux tensor
    packs sm0/sm1 on partition halves plus a duplicated bias block). Each
    chunk is one psum accumulation group: start=True only on its first
    matmul, stop=True on the closing bias matmul.
  - psum chunk -> fp16 SBUF evict (DVE / Pool), then an xbar DMA transpose
    turns the [out, slot] columns into row-layout tiles (row j of the slot
    order lands at [j % 128, j // 128]).
  - Pass A rows land in slot order = output order: contiguous DMA to the
    fp16 out tensor. Pass B rows are combined with a single SWDGE
    dma_scatter_add (out[idx[j]] += rows1[j]) whose per-core index tensor
    maps each k=1 slot to its token's k=0 slot. Pad rows are exactly zero
    and are pointed at pass-A pad slots (never at a real row: on hardware
    the DRAM read-modify-write of a pad's +0 add can race a real add).
  - Host unshards: out[assign[c][i]] = dev_out[c][pos0[c][i]], upcast f32.
"""

import numpy as np

N, K, IN, OUT, NUM_BANKS = 8192, 2, 128, 128, 64
NCORES = 8
NLOC = N // NCORES
P = 128
PSUM_FREE = 512


def _routing_plan(sel_all):
    """Greedy token->core assignment balancing per-bank counts for both the
    k=0 and k=1 pair distributions. Returns (assign, caps0, offs0, caps1,
    offs1, Cap) with a shared capacity plan (SPMD) and equal Cap per pass."""
    sel_all = np.asarray(sel_all).astype(np.int64)
    g0 = np.bincount(sel_all[:, 0], minlength=NUM_BANKS)
    g1 = np.bincount(sel_all[:, 1], minlength=NUM_BANKS)
    ideal0 = (g0 + NCORES - 1) // NCORES
    ideal1 = (g1 + NCORES - 1) // NCORES
    c0 = np.zeros((NCORES, NUM_BANKS), dtype=np.int64)
    c1 = np.zeros((NCORES, NUM_BANKS), dtype=np.int64)
    fill = np.zeros(NCORES, dtype=np.int64)
    assign_lists = [[] for _ in range(NCORES)]
    for n in range(N):
        b0, b1 = int(sel_all[n, 0]), int(sel_all[n, 1])
        best, best_key = -1, None
        for c in range(NCORES):
            if fill[c] >= NLOC:
                continue
            over = max(0, c0[c, b0] + 1 - ideal0[b0]) + \
                max(0, c1[c, b1] + 1 - ideal1[b1])
            key = (over, c0[c, b0] + c1[c, b1], fill[c])
            if best < 0 or key < best_key:
                best, best_key = c, key
        c0[best, b0] += 1
        c1[best, b1] += 1
        fill[best] += 1
        assign_lists[best].append(n)
    assign = np.array(assign_lists, dtype=np.int64)

    caps0 = c0.max(axis=0).astype(np.int64)
    caps1 = c1.max(axis=0).astype(np.int64)
    Cap = -(-max(int(caps0.sum()), int(caps1.sum())) // P) * P

    def pad_to(caps, target):
        pad = target - int(caps.sum())
        for i in range(pad):
            caps[i % NUM_BANKS] += 1
        offs = np.concatenate([[0], np.cumsum(caps)[:-1]]).astype(np.int64)
        return caps, offs

    caps0, offs0 = pad_to(caps0, Cap)
    caps1, offs1 = pad_to(caps1, Cap)
    return assign, caps0, offs0, caps1, offs1, Cap


def _wrap_idx(flat_idx):
    """Wrap a flat int16 index list into the [128, n//16] SWDGE layout."""
    n = flat_idx.shape[0]
    assert n % 16 == 0
    w = flat_idx.reshape(n // 16, 16).T.astype(np.int16)
    return np.tile(w, (8, 1))


def _segments(caps, offs):
    """Per-bank psum column segments split at PSUM_FREE boundaries, grouped
    by chunk. Returns {chunk: [(bank, col_start, width), ...]}."""
    by_chunk = {}
    for b in range(NUM_BANKS):
        cb, ob = int(caps[b]), int(offs[b])
        while cb > 0:
            room = PSUM_FREE - (ob % PSUM_FREE)
            w = min(cb, room)
            by_chunk.setdefault(ob // PSUM_FREE, []).append((b, ob, w))
            ob += w
            cb -= w
    return by_chunk


def _build_program(caps0, offs0, caps1, offs1, Cap):
    import concourse.bacc as bacc
    import concourse.tile as tile
    from concourse import mybir, library_config
    from concourse.tile import add_dep_helper

    f32 = mybir.dt.float32
    f16 = mybir.dt.float16
    i16 = mybir.dt.int16

    nblk = Cap // P
    nch = (Cap + PSUM_FREE - 1) // PSUM_FREE

    nc = bacc.Bacc(None, target_bir_lowering=False, debug=False)

    x0_d = nc.declare_dram_parameter("x0", [P, Cap], f16, isOutput=False)
    x1_d = nc.declare_dram_parameter("x1", [P, Cap], f16, isOutput=False)
    w_d = nc.declare_dram_parameter("wts", [P, NUM_BANKS * OUT], f16,
                                    isOutput=False)
    aux_d = nc.declare_dram_parameter("aux", [P, Cap + P], f16, isOutput=False)
    idx_d = nc.declare_dram_parameter("sidx", [P, Cap // 16], i16,
                                      isOutput=False)
    out_d = nc.declare_dram_parameter("out", [Cap, OUT], f16, isOutput=True)

    def segs_by_bank(caps, offs):
        by_bank = {}
        for b in range(NUM_BANKS):
            cb, ob = int(caps[b]), int(offs[b])
            while cb > 0:
                room = PSUM_FREE - (ob % PSUM_FREE)
                w = min(cb, room)
                by_bank.setdefault(b, []).append((ob, w))
                ob += w
                cb -= w
        return by_bank

    segs0_by_bank = segs_by_bank(caps0, offs0)
    segs1_by_bank = segs_by_bank(caps1, offs1)

    # weight DMA pieces: cut at every psum chunk boundary (both passes) and
    # subdivide so matmuls can start as soon as each piece lands
    cuts = {0, NUM_BANKS}
    for i in range(1, nch):
        cuts.add(next(b for b in range(NUM_BANKS) if offs0[b] >= i * PSUM_FREE))
        cuts.add(next(b for b in range(NUM_BANKS) if offs1[b] >= i * PSUM_FREE))
    cuts = sorted(cuts)
    wcut = [0]
    for lo, hi in zip(cuts[:-1], cuts[1:]):
        step = max(6, (hi - lo + 1) // 2)
        b = lo
        while b < hi:
            b = min(b + step, hi)
            wcut.append(b)

    with tile.TileContext(nc) as tc:
        with (
            tc.tile_pool(name="const", bufs=1) as cpool,
            tc.tile_pool(name="big", bufs=1) as bigpool,
            tc.tile_pool(name="psA", bufs=1, space="PSUM") as psA,
            tc.tile_pool(name="psB", bufs=1, space="PSUM") as psB,
        ):
            libload = nc.gpsimd.load_library(library_config.mlp)

            # input DMAs on SP; weight chunks on ACT/ACT/Pool
            aux_sb = cpool.tile([P, Cap + P], f16)
            nc.sync.dma_start(out=aux_sb[:], in_=aux_d.ap())
            xs0 = cpool.tile([P, Cap], f16)
            nc.sync.dma_start(out=xs0[:], in_=x0_d.ap())
            xs1 = cpool.tile([P, Cap], f16)
            nc.sync.dma_start(out=xs1[:], in_=x1_d.ap())
            idx_sb = cpool.tile([P, Cap // 16], i16)
            nc.sync.dma_start(out=idx_sb[:], in_=idx_d.ap())

            w_sb = cpool.tile([P, NUM_BANKS * OUT], f16)
            for i in range(len(wcut) - 1):
                lo, hi = wcut[i] * OUT, wcut[i + 1] * OUT
                eng = nc.scalar if i % 2 == 0 else nc.gpsimd
                eng.dma_start(out=w_sb[:, lo:hi], in_=w_d[:, lo:hi])

            pA, pB = [], []
            for i in range(nch):
                pt = psA.tile([P, PSUM_FREE], f32, tag=f"pA{i}", name=f"pA{i}")
                pA.append(pt)
            for i in range(nch):
                pt = psB.tile([P, PSUM_FREE], f32, tag=f"pB{i}", name=f"pB{i}")
                pB.append(pt)

            y0T = bigpool.tile([P, Cap], f16, tag="y0T")
            y1T = bigpool.tile([P, Cap], f16, tag="y1T")
            rows0c = []
            for ci in range(nch):
                wch = min(PSUM_FREE, Cap - ci * PSUM_FREE)
                rt = bigpool.tile([P, wch // P, OUT], f16, tag=f"rows0_{ci}",
                                  name=f"rows0_{ci}")
                rows0c.append(rt)
            rows1 = bigpool.tile([P, nblk, OUT], f16, tag="rows1")

            def finish_chunk(ptile, xs, yT, pbase, ci, is_a):
                lo = ci * PSUM_FREE
                wch = min(PSUM_FREE, Cap - lo)
                nc.tensor.matmul(
                    out=ptile[:, :wch],
                    lhsT=aux_sb[pbase:pbase + NUM_BANKS, Cap:Cap + P],
                    rhs=aux_sb[pbase:pbase + NUM_BANKS, lo:lo + wch],
                    start=False, stop=True)
                # evict psum -> fp16 (DVE; Pool has no PSUM read port)
                nc.vector.tensor_copy(yT[:, lo:lo + wch], ptile[:, :wch])
                blo, bn = lo // P, wch // P
                if is_a:
                    nc.sync.dma_start_transpose(
                        out=rows0c[ci][:, :, :], in_=yT[:, lo:lo + wch])
                    nc.scalar.dma_start(
                        out=out_d[lo:lo + wch].rearrange("(t p) o -> p t o",
                                                         p=P),
                        in_=rows0c[ci][:, :, :])
                else:
                    nc.sync.dma_start_transpose(
                        out=rows1[:, blo:blo + bn, :], in_=yT[:, lo:lo + wch])

            # matmuls grouped by weight piece so the PE starts as soon as
            # piece 0 lands; both passes interleave per piece. One psum
            # accumulation group per chunk: start=True on the chunk's first
            # matmul, stop=True on the closing bias matmul (finish_chunk,
            # emitted right after the chunk's last weight piece).
            started = set()
            last_bank = {}
            for key, segl in (("A", segs0_by_bank), ("B", segs1_by_bank)):
                for b, lst in segl.items():
                    for (ob, wseg) in lst:
                        ci = ob // PSUM_FREE
                        last_bank[(key, ci)] = max(
                            last_bank.get((key, ci), 0), b)
            finished = set()

            def emit_ready_chunks(done_banks):
                for ci in range(nch):
                    for key in ("A", "B"):
                        if (key, ci) in finished or \
                                last_bank[(key, ci)] >= done_banks:
                            continue
                        finished.add((key, ci))
                        if key == "A":
                            finish_chunk(pA[ci], xs0, y0T, 0, ci, True)
                        else:
                            finish_chunk(pB[ci], xs1, y1T, NUM_BANKS, ci,
                                         False)

            for i in range(len(wcut) - 1):
                for b in range(wcut[i], wcut[i + 1]):
                    for key, segl, ptl, xs in (("A", segs0_by_bank, pA, xs0),
                                               ("B", segs1_by_bank, pB, xs1)):
                        for (ob, wseg) in segl.get(b, ()):
                            ci = ob // PSUM_FREE
                            co = ob % PSUM_FREE
                            first = (key, ci) not in started
                            started.add((key, ci))
                            nc.tensor.matmul(
                                out=ptl[ci][:, co:co + wseg],
                                lhsT=w_sb[:, b * OUT:(b + 1) * OUT],
                                rhs=xs[:, ob:ob + wseg],
                                start=first, stop=False)
                emit_ready_chunks(wcut[i + 1])


            sa = nc.gpsimd.dma_scatter_add(
                out_ap=out_d.ap(),
                in_ap=rows1[:, :, :],
                idxs_ap=idx_sb[:],
                num_idxs=Cap,
                num_idxs_reg=Cap,
                elem_size=OUT,
                single_packet=Cap <= 1024,
            )
            add_dep_helper(sa.ins, libload.ins, sync=False,
                           reason="scatter-add needs mlp gpsimd library")

    return nc


def _make_in_maps(tensor, bank_weights, bank_selections, weights, bias,
                  assign, caps0, offs0, caps1, offs1, Cap):
    tensor = np.ascontiguousarray(tensor, dtype=np.float32)
    bank_weights = np.ascontiguousarray(bank_weights, dtype=np.float32)
    sel_all = np.asarray(bank_selections).astype(np.int64)
    w16 = np.ascontiguousarray(
        np.asarray(weights, dtype=np.float32).transpose(1, 0, 2)
        .reshape(IN, NUM_BANKS * OUT)).astype(np.float16)
    bias16 = np.ascontiguousarray(bias, dtype=np.float32).astype(np.float16)

    in_maps = []
    pos0_all = []
    for c in range(NCORES):
        toks = assign[c]
        sel = sel_all[toks]                      # [NLOC, K]
        bw = bank_weights[toks]                  # [NLOC, K]
        x = tensor[toks]                         # [NLOC, IN]

        def lay(k, offs):
            slot = np.zeros(NLOC, dtype=np.int64)
            fillb = offs.copy()
            for i in range(NLOC):
                b = sel[i, k]
                slot[i] = fillb[b]
                fillb[b] += 1
            xbw = np.zeros((Cap, IN), dtype=np.float32)
            xbw[slot] = x * bw[:, k:k + 1]
            sm = np.zeros((NUM_BANKS, Cap), dtype=np.float32)
            sm[sel[:, k], slot] = bw[:, k]
            return slot, np.ascontiguousarray(xbw.T).astype(np.float16), \
                sm.astype(np.float16)

        slot0, x0, sm0 = lay(0, offs0)
        slot1, x1, sm1 = lay(1, offs1)
        pos0_all.append(slot0)

        aux = np.zeros((P, Cap + P), dtype=np.float16)
        aux[:NUM_BANKS, :Cap] = sm0
        aux[NUM_BANKS:, :Cap] = sm1
        aux[:NUM_BANKS, Cap:] = bias16
        aux[NUM_BANKS:, Cap:] = bias16

        # scatter indices: k=1 slot j -> that token's k=0 slot. Pad rows are
        # all-zero but their scatter-add is a DRAM read-modify-write that can
        # race a real add to the same row on hardware — point them at pass-A
        # pad slots (rows no token reads) instead of a shared real row.
        pad0 = np.setdiff1d(np.arange(Cap, dtype=np.int64), slot0)
        assert pad0.size > 0
        reps = (Cap + pad0.size - 1) // pad0.size
        sidx = np.tile(pad0, reps)[:Cap]
        sidx[slot1] = slot0
        in_maps.append({
            "x0": x0,
            "x1": x1,
            "wts": w16,
            "aux": aux,
            "sidx": _wrap_idx(sidx.astype(np.int16)),
        })
    return in_maps, pos0_all


def kernel(tensor, bank_weights, bank_selections, weights, bias):
    tensor = np.asarray(tensor)
    bank_weights = np.asarray(bank_weights)
    bank_selections = np.asarray(bank_selections)
    weights = np.asarray(weights)
    bias = np.asarray(bias)

    assign, caps0, offs0, caps1, offs1, Cap = _routing_plan(bank_selections)
    nc = _build_program(caps0, offs0, caps1, offs1, Cap)
    in_maps, pos0_all = _make_in_maps(
        tensor, bank_weights, bank_selections, weights, bias,
        assign, caps0, offs0, caps1, offs1, Cap)

    nc.finalize()
    from concourse.bass_utils import run_bass_kernel_spmd
    try:
        res = run_bass_kernel_spmd(nc, in_maps, list(range(NCORES)))
    except Exception:
        # one retry: a previous crashed session can leave the accelerator in
        # a transient bad state that clears on the next dispatch
        import time
        time.sleep(2.0)
        res = run_bass_kernel_spmd(nc, in_maps, list(range(NCORES)))
    out = np.empty((N, OUT), dtype=np.float32)
    for c in range(NCORES):
        out[assign[c]] = res.results[c]["out"][pos0_all[c]].astype(np.float32)
    return out


# revision 19
# speedup vs baseline: 1.6912x; 1.5099x over previous
"""BankedLinear (MoE-style banked linear) Trainium2 Bass kernel.

Math: out[n] = sum_k bank_weights[n,k] * (tensor[n] @ W[sel[n,k]] + bias[sel[n,k]])
Shapes: tensor [8192,128] f32, bank_weights [8192,2] f32, bank_selections [8192,2] int,
        weights [64,128,128] f32, bias [64,128] f32 -> out [8192,128] f32.

Strategy (data parallel over tokens, weights replicated, fp16 compute):
  - 8 cores x 1024 tokens, greedily assigned so per-bank pair counts are
    balanced across cores (SPMD: one program, shared per-bank capacity plan).
  - Two sorted passes per core. Pass A processes each token's k=0 pair with
    tokens sorted by sel[:,0]; pass B processes k=1 pairs sorted by sel[:,1].
    The host pre-builds x^T column tiles in each sorted order with the
    bank_weight folded in (column s = bw[pair s] * x[token s], fp16), so a
    single fp16 matmul per bank segment computes bw*x@W. The bias term is
    added per 512-column psum chunk with one 64-contraction matmul against a
    one-hot routing matrix sm[b, s] = bw[s]*[bank(s)==b] (the aux tensor
    packs sm0/sm1 on partition halves plus a duplicated bias block). Each
    chunk is one psum accumulation group: start=True only on its first
    matmul, stop=True on the closing bias matmul.
  - psum chunk -> fp16 SBUF evict (DVE / Pool), then an xbar DMA transpose
    turns the [out, slot] columns into row-layout tiles (row j of the slot
    order lands at [j % 128, j // 128]).
  - Pass A rows land in slot order = output order: contiguous DMA to the
    fp16 out tensor. Pass B rows are combined with a single SWDGE
    dma_scatter_add (out[idx[j]] += rows1[j]) whose per-core index tensor
    maps each k=1 slot to its token's k=0 slot. Pad rows are exactly zero
    and are pointed at pass-A pad slots (never at a real row: on hardware
    the DRAM read-modify-write of a pad's +0 add can race a real add).
  - Host unshards: out[assign[c][i]] = dev_out[c][pos0[c][i]], upcast f32.
"""

import numpy as np

N, K, IN, OUT, NUM_BANKS = 8192, 2, 128, 128, 64
NCORES = 8
NLOC = N // NCORES
P = 128
PSUM_FREE = 512


def _routing_plan(sel_all):
    """Greedy token->core assignment balancing per-bank counts for both the
    k=0 and k=1 pair distributions. Returns (assign, caps0, offs0, caps1,
    offs1, Cap) with a shared capacity plan (SPMD) and equal Cap per pass."""
    sel_all = np.asarray(sel_all).astype(np.int64)
    g0 = np.bincount(sel_all[:, 0], minlength=NUM_BANKS)
    g1 = np.bincount(sel_all[:, 1], minlength=NUM_BANKS)
    ideal0 = (g0 + NCORES - 1) // NCORES
    ideal1 = (g1 + NCORES - 1) // NCORES
    c0 = np.zeros((NCORES, NUM_BANKS), dtype=np.int64)
    c1 = np.zeros((NCORES, NUM_BANKS), dtype=np.int64)
    fill = np.zeros(NCORES, dtype=np.int64)
    assign_lists = [[] for _ in range(NCORES)]
    for n in range(N):
        b0, b1 = int(sel_all[n, 0]), int(sel_all[n, 1])
        best, best_key = -1, None
        for c in range(NCORES):
            if fill[c] >= NLOC:
                continue
            over = max(0, c0[c, b0] + 1 - ideal0[b0]) + \
                max(0, c1[c, b1] + 1 - ideal1[b1])
            key = (over, c0[c, b0] + c1[c, b1], fill[c])
            if best < 0 or key < best_key:
                best, best_key = c, key
        c0[best, b0] += 1
        c1[best, b1] += 1
        fill[best] += 1
        assign_lists[best].append(n)
    assign = np.array(assign_lists, dtype=np.int64)

    caps0 = c0.max(axis=0).astype(np.int64)
    caps1 = c1.max(axis=0).astype(np.int64)
    Cap = -(-max(int(caps0.sum()), int(caps1.sum())) // P) * P

    def pad_to(caps, target):
        pad = target - int(caps.sum())
        for i in range(pad):
            caps[i % NUM_BANKS] += 1
        offs = np.concatenate([[0], np.cumsum(caps)[:-1]]).astype(np.int64)
        return caps, offs

    caps0, offs0 = pad_to(caps0, Cap)
    caps1, offs1 = pad_to(caps1, Cap)
    return assign, caps0, offs0, caps1, offs1, Cap


def _wrap_idx(flat_idx):
    """Wrap a flat int16 index list into the [128, n//16] SWDGE layout."""
    n = flat_idx.shape[0]
    assert n % 16 == 0
    w = flat_idx.reshape(n // 16, 16).T.astype(np.int16)
    return np.tile(w, (8, 1))


def _segments(caps, offs):
    """Per-bank psum column segments split at PSUM_FREE boundaries, grouped
    by chunk. Returns {chunk: [(bank, col_start, width), ...]}."""
    by_chunk = {}
    for b in range(NUM_BANKS):
        cb, ob = int(caps[b]), int(offs[b])
        while cb > 0:
            room = PSUM_FREE - (ob % PSUM_FREE)
            w = min(cb, room)
            by_chunk.setdefault(ob // PSUM_FREE, []).append((b, ob, w))
            ob += w
            cb -= w
    return by_chunk


def _build_program(caps0, offs0, caps1, offs1, Cap):
    import concourse.bacc as bacc
    import concourse.tile as tile
    from concourse import mybir, library_config
    from concourse.tile import add_dep_helper

    f32 = mybir.dt.float32
    f16 = mybir.dt.float16
    i16 = mybir.dt.int16

    nblk = Cap // P
    nch = (Cap + PSUM_FREE - 1) // PSUM_FREE

    nc = bacc.Bacc(None, target_bir_lowering=False, debug=False)

    # inp1 = aux (sm0/sm1 + bias) || xs0; inp2 = xs1 || idx-bits
    w_d = nc.declare_dram_parameter("wts", [P, NUM_BANKS * OUT], f16,
                                    isOutput=False)
    inp1_d = nc.declare_dram_parameter("inp1", [P, (Cap + P) + Cap], f16,
                                       isOutput=False)
    inp2_d = nc.declare_dram_parameter("inp2", [P, Cap + Cap // 16], f16,
                                       isOutput=False)
    out_d = nc.declare_dram_parameter("out", [Cap, OUT], f16, isOutput=True)

    def segs_by_bank(caps, offs):
        by_bank = {}
        for b in range(NUM_BANKS):
            cb, ob = int(caps[b]), int(offs[b])
            while cb > 0:
                room = PSUM_FREE - (ob % PSUM_FREE)
                w = min(cb, room)
                by_bank.setdefault(b, []).append((ob, w))
                ob += w
                cb -= w
        return by_bank

    segs0_by_bank = segs_by_bank(caps0, offs0)
    segs1_by_bank = segs_by_bank(caps1, offs1)

    # weight DMA pieces: cut at every psum chunk boundary (both passes) and
    # subdivide so matmuls can start as soon as each piece lands
    cuts = {0, NUM_BANKS}
    for i in range(1, nch):
        cuts.add(next(b for b in range(NUM_BANKS) if offs0[b] >= i * PSUM_FREE))
        cuts.add(next(b for b in range(NUM_BANKS) if offs1[b] >= i * PSUM_FREE))
    cuts = sorted(cuts)
    wcut = [0]
    for lo, hi in zip(cuts[:-1], cuts[1:]):
        step = max(6, (hi - lo + 1) // 2)
        b = lo
        while b < hi:
            b = min(b + step, hi)
            wcut.append(b)

    with tile.TileContext(nc) as tc:
        with (
            tc.tile_pool(name="const", bufs=1) as cpool,
            tc.tile_pool(name="big", bufs=1) as bigpool,
            tc.tile_pool(name="psA", bufs=1, space="PSUM") as psA,
            tc.tile_pool(name="psB", bufs=1, space="PSUM") as psB,
        ):
            libload = nc.gpsimd.load_library(library_config.mlp)

            # input DMAs on SP (packed: 2 ops); weight pieces on ACT/Pool
            inp1_sb = cpool.tile([P, (Cap + P) + Cap], f16)
            nc.sync.dma_start(out=inp1_sb[:], in_=inp1_d.ap())
            inp2_sb = cpool.tile([P, Cap + Cap // 16], f16)
            nc.sync.dma_start(out=inp2_sb[:], in_=inp2_d.ap())
            aux_sb = inp1_sb[:, :Cap + P]
            xs0 = inp1_sb[:, Cap + P:]
            xs1 = inp2_sb[:, :Cap]
            idx_sb = inp2_sb[:, Cap:].bitcast(i16)

            w_sb = cpool.tile([P, NUM_BANKS * OUT], f16)
            for i in range(len(wcut) - 1):
                lo, hi = wcut[i] * OUT, wcut[i + 1] * OUT
                eng = nc.scalar if i % 2 == 0 else nc.gpsimd
                eng.dma_start(out=w_sb[:, lo:hi], in_=w_d[:, lo:hi])

            pA, pB = [], []
            for i in range(nch):
                pt = psA.tile([P, PSUM_FREE], f32, tag=f"pA{i}", name=f"pA{i}")
                pA.append(pt)
            for i in range(nch):
                pt = psB.tile([P, PSUM_FREE], f32, tag=f"pB{i}", name=f"pB{i}")
                pB.append(pt)

            y0T = bigpool.tile([P, Cap], f16, tag="y0T")
            y1T = bigpool.tile([P, Cap], f16, tag="y1T")
            rows0c = []
            for ci in range(nch):
                wch = min(PSUM_FREE, Cap - ci * PSUM_FREE)
                rt = bigpool.tile([P, wch // P, OUT], f16, tag=f"rows0_{ci}",
                                  name=f"rows0_{ci}")
                rows0c.append(rt)
            rows1 = bigpool.tile([P, nblk, OUT], f16, tag="rows1")

            def finish_chunk(ptile, xs, yT, pbase, ci, is_a):
                lo = ci * PSUM_FREE
                wch = min(PSUM_FREE, Cap - lo)
                nc.tensor.matmul(
                    out=ptile[:, :wch],
                    lhsT=aux_sb[pbase:pbase + NUM_BANKS, Cap:Cap + P],
                    rhs=aux_sb[pbase:pbase + NUM_BANKS, lo:lo + wch],
                    start=False, stop=True)
                # evict psum -> fp16 on DVE (only DVE/ACT can read PSUM;
                # keeping ACT compute-free avoids its LoadActFuncSet)
                nc.vector.tensor_copy(yT[:, lo:lo + wch], ptile[:, :wch])
                blo, bn = lo // P, wch // P
                if is_a:
                    # transpose and out-write ride the same SP HWDGE ring:
                    # FIFO order makes the RAW wait free and avoids the
                    # cyclic DMAHW-sem coarsening across rings
                    nc.sync.dma_start_transpose(
                        out=rows0c[ci][:, :, :], in_=yT[:, lo:lo + wch])
                    nc.sync.dma_start(
                        out=out_d[lo:lo + wch].rearrange("(t p) o -> p t o",
                                                         p=P),
                        in_=rows0c[ci][:, :, :])
                else:
                    nc.sync.dma_start_transpose(
                        out=rows1[:, blo:blo + bn, :], in_=yT[:, lo:lo + wch])

            # matmuls grouped by weight piece so the PE starts as soon as
            # piece 0 lands; both passes interleave per piece. One psum
            # accumulation group per chunk: start=True on the chunk's first
            # matmul, stop=True on the closing bias matmul (finish_chunk,
            # emitted right after the chunk's last weight piece).
            started = set()
            last_bank = {}
            for key, segl in (("A", segs0_by_bank), ("B", segs1_by_bank)):
                for b, lst in segl.items():
                    for (ob, wseg) in lst:
                        ci = ob // PSUM_FREE
                        last_bank[(key, ci)] = max(
                            last_bank.get((key, ci), 0), b)
            finished = set()

            def emit_ready_chunks(done_banks):
                for ci in range(nch):
                    for key in ("A", "B"):
                        if (key, ci) in finished or \
                                last_bank[(key, ci)] >= done_banks:
                            continue
                        finished.add((key, ci))
                        if key == "A":
                            finish_chunk(pA[ci], xs0, y0T, 0, ci, True)
                        else:
                            finish_chunk(pB[ci], xs1, y1T, NUM_BANKS, ci,
                                         False)

            for i in range(len(wcut) - 1):
                for b in range(wcut[i], wcut[i + 1]):
                    for key, segl, ptl, xs in (("A", segs0_by_bank, pA, xs0),
                                               ("B", segs1_by_bank, pB, xs1)):
                        for (ob, wseg) in segl.get(b, ()):
                            ci = ob // PSUM_FREE
                            co = ob % PSUM_FREE
                            first = (key, ci) not in started
                            started.add((key, ci))
                            nc.tensor.matmul(
                                out=ptl[ci][:, co:co + wseg],
                                lhsT=w_sb[:, b * OUT:(b + 1) * OUT],
                                rhs=xs[:, ob:ob + wseg],
                                start=first, stop=False)
                emit_ready_chunks(wcut[i + 1])


            dma_sem = nc.alloc_semaphore("swdge_dma")
            sa = nc.gpsimd.dma_scatter_add(
                out_ap=out_d.ap(),
                in_ap=rows1[:, :, :],
                idxs_ap=idx_sb,
                num_idxs=Cap,
                num_idxs_reg=Cap,
                elem_size=OUT,
                single_packet=Cap <= 1024,
                prepare_only=True,
                sem=dma_sem,
            )
            add_dep_helper(sa.ins, libload.ins, sync=False,
                           reason="scatter-add needs mlp gpsimd library")
            nc.gpsimd.trigger_dma(count=None)

    return nc


def _make_in_maps(tensor, bank_weights, bank_selections, weights, bias,
                  assign, caps0, offs0, caps1, offs1, Cap):
    tensor = np.ascontiguousarray(tensor, dtype=np.float32)
    bank_weights = np.ascontiguousarray(bank_weights, dtype=np.float32)
    sel_all = np.asarray(bank_selections).astype(np.int64)
    w16 = np.ascontiguousarray(
        np.asarray(weights, dtype=np.float32).transpose(1, 0, 2)
        .reshape(IN, NUM_BANKS * OUT)).astype(np.float16)
    bias16 = np.ascontiguousarray(bias, dtype=np.float32).astype(np.float16)

    in_maps = []
    pos0_all = []
    for c in range(NCORES):
        toks = assign[c]
        sel = sel_all[toks]                      # [NLOC, K]
        bw = bank_weights[toks]                  # [NLOC, K]
        x = tensor[toks]                         # [NLOC, IN]

        def lay(k, offs):
            slot = np.zeros(NLOC, dtype=np.int64)
            fillb = offs.copy()
            for i in range(NLOC):
                b = sel[i, k]
                slot[i] = fillb[b]
                fillb[b] += 1
            xbw = np.zeros((Cap, IN), dtype=np.float32)
            xbw[slot] = x * bw[:, k:k + 1]
            sm = np.zeros((NUM_BANKS, Cap), dtype=np.float32)
            sm[sel[:, k], slot] = bw[:, k]
            return slot, np.ascontiguousarray(xbw.T).astype(np.float16), \
                sm.astype(np.float16)

        slot0, x0, sm0 = lay(0, offs0)
        slot1, x1, sm1 = lay(1, offs1)
        pos0_all.append(slot0)

        aux = np.zeros((P, Cap + P), dtype=np.float16)
        aux[:NUM_BANKS, :Cap] = sm0
        aux[NUM_BANKS:, :Cap] = sm1
        aux[:NUM_BANKS, Cap:] = bias16
        aux[NUM_BANKS:, Cap:] = bias16

        # scatter indices: k=1 slot j -> that token's k=0 slot. Pad rows are
        # all-zero but their scatter-add is a DRAM read-modify-write that can
        # race a real add to the same row on hardware — point them at pass-A
        # pad slots (rows no token reads) instead of a shared real row.
        pad0 = np.setdiff1d(np.arange(Cap, dtype=np.int64), slot0)
        assert pad0.size > 0
        reps = (Cap + pad0.size - 1) // pad0.size
        sidx = np.tile(pad0, reps)[:Cap]
        sidx[slot1] = slot0
        inp1 = np.concatenate([aux, x0], axis=1)
        idxb = _wrap_idx(sidx.astype(np.int16)).view(np.float16)
        inp2 = np.concatenate([x1, idxb], axis=1)
        in_maps.append({
            "wts": w16,
            "inp1": np.ascontiguousarray(inp1),
            "inp2": np.ascontiguousarray(inp2),
        })
    return in_maps, pos0_all


def kernel(tensor, bank_weights, bank_selections, weights, bias):
    tensor = np.asarray(tensor)
    bank_weights = np.asarray(bank_weights)
    bank_selections = np.asarray(bank_selections)
    weights = np.asarray(weights)
    bias = np.asarray(bias)

    assign, caps0, offs0, caps1, offs1, Cap = _routing_plan(bank_selections)
    nc = _build_program(caps0, offs0, caps1, offs1, Cap)
    in_maps, pos0_all = _make_in_maps(
        tensor, bank_weights, bank_selections, weights, bias,
        assign, caps0, offs0, caps1, offs1, Cap)

    nc.finalize()
    from concourse.bass_utils import run_bass_kernel_spmd
    try:
        res = run_bass_kernel_spmd(nc, in_maps, list(range(NCORES)))
    except Exception:
        # one retry: a previous crashed session can leave the accelerator in
        # a transient bad state that clears on the next dispatch
        import time
        time.sleep(2.0)
        res = run_bass_kernel_spmd(nc, in_maps, list(range(NCORES)))
    out = np.empty((N, OUT), dtype=np.float32)
    for c in range(NCORES):
        out[assign[c]] = res.results[c]["out"][pos0_all[c]].astype(np.float32)
    return out
